# revision 12
# baseline (speedup 1.0000x reference)
"""GroupedQueryAttention (B=1, T=2048, D=4096, 32 q-heads / 8 kv-heads, hd=128)
on 8 trn2 NeuronCores.

Sharding: kv-head parallel — core c owns kv head c and its 4 query heads.
v3: 16-bit matmuls (fp16 on the q/k score path and projections for mantissa,
bf16 on the exp/value path for range), chunk-pipelined schedule with one
AllGather per 512-token chunk overlapped two chunks deep, wo matmul
column-parallel so no AllReduce is needed.  Causal attention in transposed
[k, q] score layout (softmax without max-subtraction: fp32 exp can't
overflow at these score magnitudes).
"""
import sys

sys.path.insert(0, "/opt/trn_rl_repo")

import numpy as np

import concourse.bacc as bacc
import concourse.tile as tile
from concourse import mybir
from concourse.bass_utils import run_bass_kernel_spmd
from concourse.masks import make_identity

N_CORES = 8
T = 2048
DIM = 4096
HD = 128
NH = 32
NKV = 8
NREP = NH // NKV  # 4 query heads per core
NCHUNK = T // 512  # 4 chunks of 512 along T
NKT = DIM // 128  # 32 contraction tiles for the projections
NTT = T // 128  # 16 row tiles for the wo matmul
F32 = mybir.dt.float32
FP16 = mybir.dt.float16
BF16 = mybir.dt.bfloat16
SCALE = 1.0 / float(np.sqrt(HD))

_cached = {}


def _build_kernel():
    if "nc" in _cached:
        return _cached["nc"]

    nc = bacc.Bacc("TRN2", target_bir_lowering=False)

    xT = nc.dram_tensor("xT", [DIM, T], FP16, kind="ExternalInput")
    cos2 = nc.dram_tensor("cos2", [128, T], F32, kind="ExternalInput")
    sin2 = nc.dram_tensor("sin2", [128, T], F32, kind="ExternalInput")
    masks = nc.dram_tensor("masks", [128, 4 * 512], BF16, kind="ExternalInput")
    # weights pre-packed on host: [128, n*m] with partition-contiguous rows
    wq_p = nc.dram_tensor("wq_p", [128, NKT * NREP * HD], FP16, kind="ExternalInput")
    wk_p = nc.dram_tensor("wk_p", [128, NKT * HD], FP16, kind="ExternalInput")
    wv_p = nc.dram_tensor("wv_p", [128, NKT * HD], FP16, kind="ExternalInput")
    wo_p = nc.dram_tensor("wo_p", [128, NKT * NREP * HD], FP16, kind="ExternalInput")
    out = nc.dram_tensor("out", [T, NREP * HD], F32, kind="ExternalOutput")

    y_in = [
        nc.dram_tensor(f"y_in{qc}", [NREP * HD, 512], FP16, kind="Internal")
        for qc in range(NCHUNK)
    ]
    y_all = [
        nc.dram_tensor(
            f"y_all{qc}", [DIM, 512], FP16, kind="Internal", addr_space="Shared"
        )
        for qc in range(NCHUNK)
    ]

    with tile.TileContext(nc) as tc:
        with (
            tc.tile_pool(name="consts", bufs=1) as consts,
            tc.tile_pool(name="weights", bufs=1) as weights,
            tc.tile_pool(name="acts", bufs=1) as acts,
            tc.tile_pool(name="ybuf", bufs=1) as ybuf,
            tc.tile_pool(name="stream", bufs=8) as stream,
            tc.tile_pool(name="work", bufs=2) as work,
            tc.tile_pool(name="lrec", bufs=2) as lrec,
            tc.tile_pool(name="expp", bufs=4) as expp,
            tc.tile_pool(name="outp", bufs=3) as outp,
            tc.tile_pool(name="psum", bufs=7, space="PSUM") as psum,
            tc.tile_pool(name="psumv", bufs=1, space="PSUM") as psumv,
        ):
            # ---------- resident weights, kt-sliced so proj(0) starts early ----
            wq_r = wq_p.rearrange("p (n m) -> p n m", n=NKT)
            wq_sbs = []
            wq_sbs.append(weights.tile([128, 8, NREP * HD], FP16, tag="wq0", name="wq_t0"))
            wk_sb = weights.tile([128, NKT, HD], FP16, tag="wk")
            wv_sb = weights.tile([128, NKT, HD], FP16, tag="wv")
            wk_r = wk_p.rearrange("p (n m) -> p n m", n=NKT)
            wv_r = wv_p.rearrange("p (n m) -> p n m", n=NKT)
            # interleave fine slices of wq0/wk/wv across many DMA queues
            for j in range(4):
                nc.sync.dma_start(
                    out=wq_sbs[0][:, 2 * j:2 * (j + 1), :],
                    in_=wq_r[:, 2 * j:2 * (j + 1), :],
                )
                nc.sync.dma_start(
                    out=wk_sb[:, 8 * j:8 * (j + 1), :],
                    in_=wk_r[:, 8 * j:8 * (j + 1), :],
                )
                nc.sync.dma_start(
                    out=wv_sb[:, 8 * j:8 * (j + 1), :],
                    in_=wv_r[:, 8 * j:8 * (j + 1), :],
                )
            for s in range(1, 4):
                wq_sbs.append(
                    weights.tile(
                        [128, 8, NREP * HD], FP16, tag=f"wq{s}", name=f"wq_t{s}"
                    )
                )
                for j in range(2):
                    nc.sync.dma_start(
                        out=wq_sbs[s][:, 4 * j:4 * (j + 1), :],
                        in_=wq_r[:, 8 * s + 4 * j:8 * s + 4 * (j + 1), :],
                    )
            wo_sb = weights.tile([128, NKT, NREP * HD], FP16, tag="wo")

            # ---------- constants (needed only once rope starts) ----------
            cos_sb = consts.tile([128, T], F32, tag="cos")
            nc.sync.dma_start(out=cos_sb, in_=cos2[:, :])
            sin_sb = consts.tile([128, T], F32, tag="sin")
            nc.sync.dma_start(out=sin_sb, in_=sin2[:, :])
            mask_sb = consts.tile([128, 4 * 512], BF16, tag="mask")
            nc.sync.dma_start(out=mask_sb, in_=masks[:, :])
            ones_col = consts.tile([128, 1], F32, tag="onesc")
            nc.vector.memset(ones_col, 1.0)
            ones_row = consts.tile([1, 128], BF16, tag="onesr")
            nc.vector.memset(ones_row, 1.0)
            ident = consts.tile([128, 128], BF16, tag="ident")
            make_identity(nc, ident)

            # activations that live through the attention phase
            qT_sb = acts.tile([128, NREP, T], FP16, tag="qt")
            kT_sb = acts.tile([128, T], FP16, tag="kt")
            vkd_sb = acts.tile([128, NTT, HD], BF16, tag="vkd")

            def proj_chunk(qc):
                """QKV projections + rope for token chunk qc."""
                cs = slice(512 * qc, 512 * (qc + 1))
                q_ps = [
                    psum.tile([128, 512], F32, tag="bank", name=f"qps{qc}_{h}")
                    for h in range(NREP)
                ]
                k_ps = psum.tile([128, 512], F32, tag="bank", name=f"kps{qc}")
                v_ps = psum.tile([128, 512], F32, tag="bank", name=f"vps{qc}")
                for kt in range(NKT):
                    xt = stream.tile([128, 512], FP16, tag="xt")
                    nc.sync.dma_start(
                        out=xt, in_=xT[128 * kt:128 * (kt + 1), cs]
                    )
                    st = kt == 0
                    sp = kt == NKT - 1
                    for h in range(NREP):
                        nc.tensor.matmul(
                            q_ps[h],
                            lhsT=wq_sbs[kt // 8][:, kt % 8, 128 * h:128 * (h + 1)],
                            rhs=xt,
                            start=st,
                            stop=sp,
                        )
                    nc.tensor.matmul(
                        k_ps, lhsT=wk_sb[:, kt, :], rhs=xt, start=st, stop=sp
                    )
                    nc.tensor.matmul(
                        v_ps, lhsT=wv_sb[:, kt, :], rhs=xt, start=st, stop=sp
                    )

                # rope: k first (unblocks h=0 scores), then the 4 q heads
                for h in [NREP, 0, 1, 2, 3]:
                    p = q_ps[h] if h < NREP else k_ps
                    dst = qT_sb[:, h, cs] if h < NREP else kT_sb[:, cs]
                    sw = work.tile([128, 512], F32, tag="sw")
                    nc.scalar.copy(sw[0:64, :], p[64:128, :])
                    nc.scalar.copy(sw[64:128, :], p[0:64, :])
                    rtmp = work.tile([128, 512], F32, tag="ropetmp")
                    # dst = p * cos + sw * (+-sin), fp16 conversion on the add
                    nc.vector.tensor_mul(rtmp, p, cos_sb[:, cs])
                    nc.vector.tensor_mul(sw, sw, sin_sb[:, cs])
                    nc.vector.tensor_add(dst, rtmp, sw)

                # v computed in [hd, T] layout; transpose 128x128 blocks to [k, hd]
                v_sb = work.tile([128, 512], BF16, tag="vsb")
                nc.scalar.copy(v_sb, v_ps)
                for s in range(4):
                    vt_ps = psumv.tile(
                        [128, 128], BF16, tag="vtbank", name=f"vt{qc}_{s}"
                    )
                    nc.tensor.transpose(
                        vt_ps, v_sb[:, 128 * s:128 * (s + 1)], ident
                    )
                    nc.scalar.copy(vkd_sb[:, 4 * qc + s, :], vt_ps)

            def attn_chunk(qc):
                """Causal attention for all 4 heads on chunk qc."""
                cs = slice(512 * qc, 512 * (qc + 1))
                nkt = 4 * qc + 4  # causal: k tiles 0 .. 4*qc+3
                for h in range(NREP):
                    yT_ps = psum.tile(
                        [128, 512], F32, tag="bank", name=f"yps{qc}_{h}"
                    )
                    l_acc = lrec.tile([128, 512], F32, tag="lacc")
                    nc.vector.memset(l_acc, 0.0)
                    for kt in range(nkt):
                        sT_ps = psum.tile(
                            [128, 512], F32, tag="bank", name=f"sps{qc}_{h}_{kt}"
                        )
                        nc.tensor.matmul(
                            sT_ps,
                            lhsT=kT_sb[:, 128 * kt:128 * (kt + 1)],
                            rhs=qT_sb[:, h, cs],
                            start=True,
                            stop=True,
                        )
                        e_sb = expp.tile([128, 512], BF16, tag="exp")
                        nc.scalar.activation(
                            e_sb, sT_ps, mybir.ActivationFunctionType.Exp,
                            scale=SCALE,
                        )
                        d = kt - 4 * qc
                        if d >= 0:  # diagonal block: zero the k > q half
                            nc.vector.tensor_mul(
                                e_sb, e_sb, mask_sb[:, 512 * d:512 * (d + 1)]
                            )
                        nc.vector.tensor_add(l_acc, l_acc, e_sb)
                        nc.tensor.matmul(
                            yT_ps,
                            lhsT=vkd_sb[:, kt, :],
                            rhs=e_sb,
                            start=(kt == 0),
                            stop=(kt == nkt - 1),
                        )
                    # softmax denominator -> reciprocal -> broadcast to 128 parts
                    l_ps = psum.tile([128, 512], F32, tag="bank", name=f"l{qc}{h}")
                    nc.tensor.matmul(
                        l_ps[0:1, :], lhsT=ones_col[:, 0:1], rhs=l_acc,
                        start=True, stop=True,
                    )
                    recip = lrec.tile([1, 512], F32, tag="recip")
                    nc.vector.reciprocal_approx_fast(recip, l_ps[0:1, :])
                    recip_bf = lrec.tile([1, 512], BF16, tag="recipbf")
                    nc.scalar.copy(recip_bf, recip)
                    bc_ps = psum.tile([128, 512], F32, tag="bank", name=f"b{qc}{h}")
                    nc.tensor.matmul(
                        bc_ps, lhsT=ones_row[0:1, :], rhs=recip_bf[0:1, :],
                        start=True, stop=True,
                    )
                    bc_sb = work.tile([128, 512], F32, tag="bc")
                    nc.scalar.copy(bc_sb, bc_ps)
                    yn_sb = work.tile([128, 512], FP16, tag="yn")
                    nc.vector.tensor_mul(yn_sb, yT_ps, bc_sb)
                    nc.sync.dma_start(
                        out=y_in[qc][128 * h:128 * (h + 1), :], in_=yn_sb
                    )
            def gather_chunk(qc):
                nc.gpsimd.collective_compute(
                    "AllGather",
                    mybir.AluOpType.bypass,
                    ins=[y_in[qc][:, :]],
                    outs=[y_all[qc][:, :]],
                    replica_groups=[list(range(N_CORES))],
                )

            def wo_chunk(qc):
                """out rows for chunk qc: needs y_all[qc] (all cores' heads)."""
                y_sb = ybuf.tile([128, NKT, 512], FP16, tag="ysb")
                y_r = y_all[qc].rearrange("(n p) m -> p n m", p=128)
                for s in range(8):
                    nc.sync.dma_start(
                        out=y_sb[:, 4 * s:4 * (s + 1), :],
                        in_=y_r[:, 4 * s:4 * (s + 1), :],
                    )
                for tt in range(4 * qc, 4 * qc + 4):
                    to = 128 * tt - 512 * qc
                    o_ps = psum.tile([128, 512], F32, tag="bank", name=f"o{tt}")
                    for kt in range(NKT):
                        nc.tensor.matmul(
                            o_ps,
                            lhsT=y_sb[:, kt, to:to + 128],
                            rhs=wo_sb[:, kt, :],
                            start=(kt == 0),
                            stop=(kt == NKT - 1),
                        )
                    o_sb = outp.tile([128, 512], F32, tag="osb")
                    nc.scalar.copy(o_sb, o_ps)
                    nc.sync.dma_start(
                        out=out[128 * tt:128 * (tt + 1), :], in_=o_sb
                    )

            # ---------- chunk-pipelined schedule ----------
            # wo(qc) is issued two chunks late so the AllGather latency is
            # covered by proj/attn of the following chunks.
            proj_chunk(0)
            attn_chunk(0)
            wo_r = wo_p.rearrange("p (n m) -> p n m", n=NKT)
            for s in range(4):
                nc.sync.dma_start(
                    out=wo_sb[:, 8 * s:8 * (s + 1), :],
                    in_=wo_r[:, 8 * s:8 * (s + 1), :],
                )
            proj_chunk(1)
            gather_chunk(0)
            attn_chunk(1)
            proj_chunk(2)
            gather_chunk(1)
            wo_chunk(0)
            attn_chunk(2)
            proj_chunk(3)
            gather_chunk(2)
            wo_chunk(1)
            attn_chunk(3)
            wo_chunk(2)
            gather_chunk(3)
            wo_chunk(3)

    nc.compile()
    _cached["nc"] = nc
    return nc


def _build_in_maps(inputs):
    return _shard_inputs(**inputs)


def _pack_w(wT, m):
    """[DIM, m] -> [128, NKT*m] with each partition's rows DRAM-contiguous."""
    return np.ascontiguousarray(
        wT.reshape(NKT, 128, m).transpose(1, 0, 2).reshape(128, NKT * m)
    )


def _shard_inputs(x, cos, sin, wq, wk, wv, wo, start_pos):
    import ml_dtypes

    bf16 = ml_dtypes.bfloat16
    x = np.asarray(x, dtype=np.float32)
    cos = np.asarray(cos, dtype=np.float32)
    sin = np.asarray(sin, dtype=np.float32)
    wq = np.asarray(wq, dtype=np.float32)
    wk = np.asarray(wk, dtype=np.float32)
    wv = np.asarray(wv, dtype=np.float32)
    wo = np.asarray(wo, dtype=np.float32)
    sp = int(start_pos)

    xT = np.ascontiguousarray(x[0].T).astype(np.float16)  # (DIM, T)
    cosT = np.ascontiguousarray(cos[sp:sp + T].T)  # (64, T)
    sinT = np.ascontiguousarray(sin[sp:sp + T].T)
    cos2 = np.concatenate([cosT, cosT], axis=0)  # (128, T)
    sin2 = np.concatenate([-sinT, sinT], axis=0)  # rotate-half signs folded in

    kk = np.arange(128)[:, None]
    qq = np.arange(512)[None, :]
    masks = np.concatenate(
        [(kk + 128 * d <= qq).astype(np.float32) for d in range(4)], axis=1
    ).astype(bf16)  # (128, 2048)

    in_maps = []
    for c in range(N_CORES):
        qrows = slice(NREP * HD * c, NREP * HD * (c + 1))
        krows = slice(HD * c, HD * (c + 1))
        in_maps.append({
            "xT": xT,
            "cos2": cos2,
            "sin2": sin2,
            "masks": masks,
            "wq_p": _pack_w(wq[qrows, :].T.astype(np.float16), NREP * HD),
            "wk_p": _pack_w(wk[krows, :].T.astype(np.float16), HD),
            "wv_p": _pack_w(wv[krows, :].T.astype(np.float16), HD),
            "wo_p": _pack_w(wo[qrows, :].T.astype(np.float16), NREP * HD),
        })
    return in_maps


def kernel(x, cos, sin, wq, wk, wv, wo, start_pos):
    in_maps = _shard_inputs(x, cos, sin, wq, wk, wv, wo, start_pos)
    nc = _build_kernel()
    res = run_bass_kernel_spmd(nc, in_maps, core_ids=list(range(N_CORES)))
    out = np.concatenate([res.results[c]["out"] for c in range(N_CORES)], axis=1)
    return out.reshape(1, T, DIM).astype(np.float32)


# revision 13
# speedup vs baseline: 1.0261x; 1.0261x over previous
"""GroupedQueryAttention (B=1, T=2048, D=4096, 32 q-heads / 8 kv-heads, hd=128)
on 8 trn2 NeuronCores.

Sharding: kv-head parallel — core c owns kv head c and its 4 query heads.
v3: 16-bit matmuls (fp16 on the q/k score path and projections for mantissa,
bf16 on the exp/value path for range), chunk-pipelined schedule with one
AllGather per 512-token chunk overlapped two chunks deep, wo matmul
column-parallel so no AllReduce is needed.  Causal attention in transposed
[k, q] score layout (softmax without max-subtraction: fp32 exp can't
overflow at these score magnitudes).
"""
import sys

sys.path.insert(0, "/opt/trn_rl_repo")

import numpy as np

import concourse.bacc as bacc
import concourse.tile as tile
from concourse import mybir
from concourse.bass_utils import run_bass_kernel_spmd
from concourse.masks import make_identity

N_CORES = 8
T = 2048
DIM = 4096
HD = 128
NH = 32
NKV = 8
NREP = NH // NKV  # 4 query heads per core
NCHUNK = T // 512  # 4 chunks of 512 along T
NKT = DIM // 128  # 32 contraction tiles for the projections
NTT = T // 128  # 16 row tiles for the wo matmul
F32 = mybir.dt.float32
FP16 = mybir.dt.float16
BF16 = mybir.dt.bfloat16
SCALE = 1.0 / float(np.sqrt(HD))

_cached = {}


def _build_kernel():
    if "nc" in _cached:
        return _cached["nc"]

    nc = bacc.Bacc("TRN2", target_bir_lowering=False)

    xT = nc.dram_tensor("xT", [DIM, T], FP16, kind="ExternalInput")
    cos2 = nc.dram_tensor("cos2", [128, T], F32, kind="ExternalInput")
    sin2 = nc.dram_tensor("sin2", [128, T], F32, kind="ExternalInput")
    masks = nc.dram_tensor("masks", [128, 4 * 512], BF16, kind="ExternalInput")
    # weights pre-packed on host: [128, n*m] with partition-contiguous rows
    wq_p = nc.dram_tensor("wq_p", [128, NKT * NREP * HD], FP16, kind="ExternalInput")
    wk_p = nc.dram_tensor("wk_p", [128, NKT * HD], FP16, kind="ExternalInput")
    wv_p = nc.dram_tensor("wv_p", [128, NKT * HD], FP16, kind="ExternalInput")
    wo_p = nc.dram_tensor("wo_p", [128, NKT * NREP * HD], FP16, kind="ExternalInput")
    out = nc.dram_tensor("out", [T, NREP * HD], F32, kind="ExternalOutput")

    y_in = [
        nc.dram_tensor(f"y_in{qc}", [NREP * HD, 512], FP16, kind="Internal")
        for qc in range(NCHUNK)
    ]
    y_all = [
        nc.dram_tensor(
            f"y_all{qc}", [DIM, 512], FP16, kind="Internal", addr_space="Shared"
        )
        for qc in range(NCHUNK)
    ]

    with tile.TileContext(nc) as tc:
        with (
            tc.tile_pool(name="consts", bufs=1) as consts,
            tc.tile_pool(name="weights", bufs=1) as weights,
            tc.tile_pool(name="acts", bufs=1) as acts,
            tc.tile_pool(name="ybuf", bufs=1) as ybuf,
            tc.tile_pool(name="stream", bufs=8) as stream,
            tc.tile_pool(name="work", bufs=2) as work,
            tc.tile_pool(name="lrec", bufs=2) as lrec,
            tc.tile_pool(name="expp", bufs=8) as expp,
            tc.tile_pool(name="outp", bufs=3) as outp,
            tc.tile_pool(name="psum", bufs=7, space="PSUM") as psum,
            tc.tile_pool(name="psumv", bufs=1, space="PSUM") as psumv,
        ):
            # ---------- resident weights, kt-sliced so proj(0) starts early ----
            wq_r = wq_p.rearrange("p (n m) -> p n m", n=NKT)
            wq_sbs = []
            wq_sbs.append(weights.tile([128, 8, NREP * HD], FP16, tag="wq0", name="wq_t0"))
            wk_sb = weights.tile([128, NKT, HD], FP16, tag="wk")
            wv_sb = weights.tile([128, NKT, HD], FP16, tag="wv")
            wk_r = wk_p.rearrange("p (n m) -> p n m", n=NKT)
            wv_r = wv_p.rearrange("p (n m) -> p n m", n=NKT)
            for j in range(2):
                nc.gpsimd.dma_start(
                    out=wq_sbs[0][:, 4 * j:4 * (j + 1), :],
                    in_=wq_r[:, 4 * j:4 * (j + 1), :],
                )
            nc.gpsimd.dma_start(out=wk_sb, in_=wk_r)
            nc.gpsimd.dma_start(out=wv_sb, in_=wv_r)
            for s in range(1, 4):
                wq_sbs.append(
                    weights.tile(
                        [128, 8, NREP * HD], FP16, tag=f"wq{s}", name=f"wq_t{s}"
                    )
                )
                nc.gpsimd.dma_start(
                    out=wq_sbs[s], in_=wq_r[:, 8 * s:8 * (s + 1), :]
                )
            wo_sb = weights.tile([128, NKT, NREP * HD], FP16, tag="wo")

            # ---------- constants (needed only once rope starts) ----------
            cos_sb = consts.tile([128, T], F32, tag="cos")
            nc.scalar.dma_start(out=cos_sb, in_=cos2[:, :])
            sin_sb = consts.tile([128, T], F32, tag="sin")
            nc.scalar.dma_start(out=sin_sb, in_=sin2[:, :])
            mask_sb = consts.tile([128, 4 * 512], BF16, tag="mask")
            nc.scalar.dma_start(out=mask_sb, in_=masks[:, :])
            ones_col = consts.tile([128, 1], F32, tag="onesc")
            nc.vector.memset(ones_col, 1.0)
            ones_row = consts.tile([1, 128], BF16, tag="onesr")
            nc.vector.memset(ones_row, 1.0)
            ident = consts.tile([128, 128], BF16, tag="ident")
            make_identity(nc, ident)

            # activations that live through the attention phase
            qT_sb = acts.tile([128, NREP, T], FP16, tag="qt")
            kT_sb = acts.tile([128, T], FP16, tag="kt")
            vkd_sb = acts.tile([128, NTT, HD], BF16, tag="vkd")

            def proj_chunk(qc):
                """QKV projections + rope for token chunk qc."""
                cs = slice(512 * qc, 512 * (qc + 1))
                q_ps = [
                    psum.tile([128, 512], F32, tag="bank", name=f"qps{qc}_{h}")
                    for h in range(NREP)
                ]
                k_ps = psum.tile([128, 512], F32, tag="bank", name=f"kps{qc}")
                v_ps = psum.tile([128, 512], F32, tag="bank", name=f"vps{qc}")
                for kt in range(NKT):
                    xt = stream.tile([128, 512], FP16, tag="xt")
                    nc.sync.dma_start(
                        out=xt, in_=xT[128 * kt:128 * (kt + 1), cs]
                    )
                    st = kt == 0
                    sp = kt == NKT - 1
                    for h in range(NREP):
                        nc.tensor.matmul(
                            q_ps[h],
                            lhsT=wq_sbs[kt // 8][:, kt % 8, 128 * h:128 * (h + 1)],
                            rhs=xt,
                            start=st,
                            stop=sp,
                        )
                    nc.tensor.matmul(
                        k_ps, lhsT=wk_sb[:, kt, :], rhs=xt, start=st, stop=sp
                    )
                    nc.tensor.matmul(
                        v_ps, lhsT=wv_sb[:, kt, :], rhs=xt, start=st, stop=sp
                    )

                # rope: k first (unblocks h=0 scores), then the 4 q heads
                for h in [NREP, 0, 1, 2, 3]:
                    p = q_ps[h] if h < NREP else k_ps
                    dst = qT_sb[:, h, cs] if h < NREP else kT_sb[:, cs]
                    sw = work.tile([128, 512], F32, tag="sw")
                    nc.scalar.copy(sw[0:64, :], p[64:128, :])
                    nc.scalar.copy(sw[64:128, :], p[0:64, :])
                    rtmp = work.tile([128, 512], F32, tag="ropetmp")
                    # dst = p * cos + sw * (+-sin), fp16 conversion on the add
                    nc.vector.tensor_mul(rtmp, p, cos_sb[:, cs])
                    nc.vector.tensor_mul(sw, sw, sin_sb[:, cs])
                    nc.vector.tensor_add(dst, rtmp, sw)

                # v computed in [hd, T] layout; transpose 128x128 blocks to [k, hd]
                v_sb = work.tile([128, 512], BF16, tag="vsb")
                nc.scalar.copy(v_sb, v_ps)
                for s in range(4):
                    vt_ps = psumv.tile(
                        [128, 128], BF16, tag="vtbank", name=f"vt{qc}_{s}"
                    )
                    nc.tensor.transpose(
                        vt_ps, v_sb[:, 128 * s:128 * (s + 1)], ident
                    )
                    nc.scalar.copy(vkd_sb[:, 4 * qc + s, :], vt_ps)

            def attn_chunk(qc):
                """Causal attention for all 4 heads on chunk qc."""
                cs = slice(512 * qc, 512 * (qc + 1))
                nkt = 4 * qc + 4  # causal: k tiles 0 .. 4*qc+3
                for h in range(NREP):
                    yT_ps = psum.tile(
                        [128, 512], F32, tag="bank", name=f"yps{qc}_{h}"
                    )
                    l_acc = lrec.tile([128, 512], F32, tag="lacc")
                    nc.vector.memset(l_acc, 0.0)
                    for kt in range(nkt):
                        sT_ps = psum.tile(
                            [128, 512], F32, tag="bank", name=f"sps{qc}_{h}_{kt}"
                        )
                        nc.tensor.matmul(
                            sT_ps,
                            lhsT=kT_sb[:, 128 * kt:128 * (kt + 1)],
                            rhs=qT_sb[:, h, cs],
                            start=True,
                            stop=True,
                        )
                        e_sb = expp.tile([128, 512], BF16, tag="exp")
                        nc.scalar.activation(
                            e_sb, sT_ps, mybir.ActivationFunctionType.Exp,
                            scale=SCALE,
                        )
                        d = kt - 4 * qc
                        if d >= 0:  # diagonal block: zero the k > q half
                            nc.vector.tensor_mul(
                                e_sb, e_sb, mask_sb[:, 512 * d:512 * (d + 1)]
                            )
                        nc.vector.tensor_add(l_acc, l_acc, e_sb)
                        nc.tensor.matmul(
                            yT_ps,
                            lhsT=vkd_sb[:, kt, :],
                            rhs=e_sb,
                            start=(kt == 0),
                            stop=(kt == nkt - 1),
                        )
                    # softmax denominator -> reciprocal -> broadcast to 128 parts
                    l_ps = psum.tile([128, 512], F32, tag="bank", name=f"l{qc}{h}")
                    nc.tensor.matmul(
                        l_ps[0:1, :], lhsT=ones_col[:, 0:1], rhs=l_acc,
                        start=True, stop=True,
                    )
                    recip = lrec.tile([1, 512], F32, tag="recip")
                    nc.vector.reciprocal_approx_fast(recip, l_ps[0:1, :])
                    recip_bf = lrec.tile([1, 512], BF16, tag="recipbf")
                    nc.scalar.copy(recip_bf, recip)
                    bc_ps = psum.tile([128, 512], F32, tag="bank", name=f"b{qc}{h}")
                    nc.tensor.matmul(
                        bc_ps, lhsT=ones_row[0:1, :], rhs=recip_bf[0:1, :],
                        start=True, stop=True,
                    )
                    bc_sb = work.tile([128, 512], F32, tag="bc")
                    nc.scalar.copy(bc_sb, bc_ps)
                    yn_sb = work.tile([128, 512], FP16, tag="yn")
                    nc.vector.tensor_mul(yn_sb, yT_ps, bc_sb)
                    nc.sync.dma_start(
                        out=y_in[qc][128 * h:128 * (h + 1), :], in_=yn_sb
                    )
            def gather_chunk(qc):
                nc.gpsimd.collective_compute(
                    "AllGather",
                    mybir.AluOpType.bypass,
                    ins=[y_in[qc][:, :]],
                    outs=[y_all[qc][:, :]],
                    replica_groups=[list(range(N_CORES))],
                )

            def wo_chunk(qc):
                """out rows for chunk qc: needs y_all[qc] (all cores' heads)."""
                y_sb = ybuf.tile([128, NKT, 512], FP16, tag="ysb")
                y_r = y_all[qc].rearrange("(n p) m -> p n m", p=128)
                for s in range(8):
                    nc.gpsimd.dma_start(
                        out=y_sb[:, 4 * s:4 * (s + 1), :],
                        in_=y_r[:, 4 * s:4 * (s + 1), :],
                    )
                for tt in range(4 * qc, 4 * qc + 4):
                    to = 128 * tt - 512 * qc
                    o_ps = psum.tile([128, 512], F32, tag="bank", name=f"o{tt}")
                    for kt in range(NKT):
                        nc.tensor.matmul(
                            o_ps,
                            lhsT=y_sb[:, kt, to:to + 128],
                            rhs=wo_sb[:, kt, :],
                            start=(kt == 0),
                            stop=(kt == NKT - 1),
                        )
                    o_sb = outp.tile([128, 512], F32, tag="osb")
                    nc.scalar.copy(o_sb, o_ps)
                    nc.sync.dma_start(
                        out=out[128 * tt:128 * (tt + 1), :], in_=o_sb
                    )

            # ---------- chunk-pipelined schedule ----------
            # wo(qc) is issued two chunks late so the AllGather latency is
            # covered by proj/attn of the following chunks.
            proj_chunk(0)
            attn_chunk(0)
            wo_r = wo_p.rearrange("p (n m) -> p n m", n=NKT)
            for s in range(4):
                nc.gpsimd.dma_start(
                    out=wo_sb[:, 8 * s:8 * (s + 1), :],
                    in_=wo_r[:, 8 * s:8 * (s + 1), :],
                )
            proj_chunk(1)
            gather_chunk(0)
            attn_chunk(1)
            proj_chunk(2)
            gather_chunk(1)
            wo_chunk(0)
            attn_chunk(2)
            proj_chunk(3)
            gather_chunk(2)
            wo_chunk(1)
            attn_chunk(3)
            wo_chunk(2)
            gather_chunk(3)
            wo_chunk(3)

    nc.compile()
    _cached["nc"] = nc
    return nc


def _build_in_maps(inputs):
    return _shard_inputs(**inputs)


def _pack_w(wT, m):
    """[DIM, m] -> [128, NKT*m] with each partition's rows DRAM-contiguous."""
    return np.ascontiguousarray(
        wT.reshape(NKT, 128, m).transpose(1, 0, 2).reshape(128, NKT * m)
    )


def _shard_inputs(x, cos, sin, wq, wk, wv, wo, start_pos):
    import ml_dtypes

    bf16 = ml_dtypes.bfloat16
    x = np.asarray(x, dtype=np.float32)
    cos = np.asarray(cos, dtype=np.float32)
    sin = np.asarray(sin, dtype=np.float32)
    wq = np.asarray(wq, dtype=np.float32)
    wk = np.asarray(wk, dtype=np.float32)
    wv = np.asarray(wv, dtype=np.float32)
    wo = np.asarray(wo, dtype=np.float32)
    sp = int(start_pos)

    xT = np.ascontiguousarray(x[0].T).astype(np.float16)  # (DIM, T)
    cosT = np.ascontiguousarray(cos[sp:sp + T].T)  # (64, T)
    sinT = np.ascontiguousarray(sin[sp:sp + T].T)
    cos2 = np.concatenate([cosT, cosT], axis=0)  # (128, T)
    sin2 = np.concatenate([-sinT, sinT], axis=0)  # rotate-half signs folded in

    kk = np.arange(128)[:, None]
    qq = np.arange(512)[None, :]
    masks = np.concatenate(
        [(kk + 128 * d <= qq).astype(np.float32) for d in range(4)], axis=1
    ).astype(bf16)  # (128, 2048)

    in_maps = []
    for c in range(N_CORES):
        qrows = slice(NREP * HD * c, NREP * HD * (c + 1))
        krows = slice(HD * c, HD * (c + 1))
        in_maps.append({
            "xT": xT,
            "cos2": cos2,
            "sin2": sin2,
            "masks": masks,
            "wq_p": _pack_w(wq[qrows, :].T.astype(np.float16), NREP * HD),
            "wk_p": _pack_w(wk[krows, :].T.astype(np.float16), HD),
            "wv_p": _pack_w(wv[krows, :].T.astype(np.float16), HD),
            "wo_p": _pack_w(wo[qrows, :].T.astype(np.float16), NREP * HD),
        })
    return in_maps


def kernel(x, cos, sin, wq, wk, wv, wo, start_pos):
    in_maps = _shard_inputs(x, cos, sin, wq, wk, wv, wo, start_pos)
    nc = _build_kernel()
    res = run_bass_kernel_spmd(nc, in_maps, core_ids=list(range(N_CORES)))
    out = np.concatenate([res.results[c]["out"] for c in range(N_CORES)], axis=1)
    return out.reshape(1, T, DIM).astype(np.float32)


# revision 14
# speedup vs baseline: 1.0487x; 1.0220x over previous
"""GroupedQueryAttention (B=1, T=2048, D=4096, 32 q-heads / 8 kv-heads, hd=128)
on 8 trn2 NeuronCores.

Sharding: kv-head parallel — core c owns kv head c and its 4 query heads.
v3: 16-bit matmuls (fp16 on the q/k score path and projections for mantissa,
bf16 on the exp/value path for range), chunk-pipelined schedule with one
AllGather per 512-token chunk overlapped two chunks deep, wo matmul
column-parallel so no AllReduce is needed.  Causal attention in transposed
[k, q] score layout (softmax without max-subtraction: fp32 exp can't
overflow at these score magnitudes).
"""
import sys

sys.path.insert(0, "/opt/trn_rl_repo")

import numpy as np

import concourse.bacc as bacc
import concourse.tile as tile
from concourse import mybir
from concourse.bass_utils import run_bass_kernel_spmd
from concourse.masks import make_identity

N_CORES = 8
T = 2048
DIM = 4096
HD = 128
NH = 32
NKV = 8
NREP = NH // NKV  # 4 query heads per core
NCHUNK = T // 512  # 4 chunks of 512 along T
NKT = DIM // 128  # 32 contraction tiles for the projections
NTT = T // 128  # 16 row tiles for the wo matmul
F32 = mybir.dt.float32
FP16 = mybir.dt.float16
BF16 = mybir.dt.bfloat16
SCALE = 1.0 / float(np.sqrt(HD))

_cached = {}


def _build_kernel():
    if "nc" in _cached:
        return _cached["nc"]

    nc = bacc.Bacc("TRN2", target_bir_lowering=False)

    xT = nc.dram_tensor("xT", [DIM, T], FP16, kind="ExternalInput")
    cos2 = nc.dram_tensor("cos2", [128, T], F32, kind="ExternalInput")
    sin2 = nc.dram_tensor("sin2", [128, T], F32, kind="ExternalInput")
    masks = nc.dram_tensor("masks", [128, 4 * 512], BF16, kind="ExternalInput")
    # weights pre-packed on host: [128, n*m] with partition-contiguous rows
    wq_p = nc.dram_tensor("wq_p", [128, NKT * NREP * HD], FP16, kind="ExternalInput")
    wk_p = nc.dram_tensor("wk_p", [128, NKT * HD], FP16, kind="ExternalInput")
    wv_p = nc.dram_tensor("wv_p", [128, NKT * HD], FP16, kind="ExternalInput")
    wo_p = nc.dram_tensor("wo_p", [128, NKT * NREP * HD], FP16, kind="ExternalInput")
    out = nc.dram_tensor("out", [T, NREP * HD], F32, kind="ExternalOutput")

    y_in = [
        nc.dram_tensor(f"y_in{qc}", [NREP * HD, 512], FP16, kind="Internal")
        for qc in range(NCHUNK)
    ]
    y_all = [
        nc.dram_tensor(
            f"y_all{qc}", [DIM, 512], FP16, kind="Internal", addr_space="Shared"
        )
        for qc in range(NCHUNK)
    ]

    with tile.TileContext(nc) as tc:
        with (
            tc.tile_pool(name="consts", bufs=1) as consts,
            tc.tile_pool(name="weights", bufs=1) as weights,
            tc.tile_pool(name="acts", bufs=1) as acts,
            tc.tile_pool(name="ybuf", bufs=1) as ybuf,
            tc.tile_pool(name="stream", bufs=8) as stream,
            tc.tile_pool(name="work", bufs=2) as work,
            tc.tile_pool(name="lrec", bufs=2) as lrec,
            tc.tile_pool(name="expp", bufs=8) as expp,
            tc.tile_pool(name="outp", bufs=3) as outp,
            tc.tile_pool(name="psum", bufs=7, space="PSUM") as psum,
            tc.tile_pool(name="psumv", bufs=1, space="PSUM") as psumv,
        ):
            # ---------- resident weights, kt-sliced so proj(0) starts early ----
            wq_r = wq_p.rearrange("p (n m) -> p n m", n=NKT)
            wq_sbs = []
            wq_sbs.append(weights.tile([128, 8, NREP * HD], FP16, tag="wq0", name="wq_t0"))
            wk_sb = weights.tile([128, NKT, HD], FP16, tag="wk")
            wv_sb = weights.tile([128, NKT, HD], FP16, tag="wv")
            wk_r = wk_p.rearrange("p (n m) -> p n m", n=NKT)
            wv_r = wv_p.rearrange("p (n m) -> p n m", n=NKT)
            for j in range(2):
                nc.gpsimd.dma_start(
                    out=wq_sbs[0][:, 4 * j:4 * (j + 1), :],
                    in_=wq_r[:, 4 * j:4 * (j + 1), :],
                )
            nc.gpsimd.dma_start(out=wk_sb, in_=wk_r)
            nc.gpsimd.dma_start(out=wv_sb, in_=wv_r)
            for s in range(1, 4):
                wq_sbs.append(
                    weights.tile(
                        [128, 8, NREP * HD], FP16, tag=f"wq{s}", name=f"wq_t{s}"
                    )
                )
                nc.gpsimd.dma_start(
                    out=wq_sbs[s], in_=wq_r[:, 8 * s:8 * (s + 1), :]
                )
            wo_sb = weights.tile([128, NKT, NREP * HD], FP16, tag="wo")

            # ---------- constants (needed only once rope starts) ----------
            cos_sb = consts.tile([128, T], F32, tag="cos")
            nc.scalar.dma_start(out=cos_sb, in_=cos2[:, :])
            sin_sb = consts.tile([128, T], F32, tag="sin")
            nc.scalar.dma_start(out=sin_sb, in_=sin2[:, :])
            mask_sb = consts.tile([128, 4 * 512], BF16, tag="mask")
            nc.scalar.dma_start(out=mask_sb, in_=masks[:, :])
            ones_col = consts.tile([128, 1], BF16, tag="onesc")
            nc.vector.memset(ones_col, 1.0)
            ones_row = consts.tile([1, 128], BF16, tag="onesr")
            nc.vector.memset(ones_row, 1.0)
            ident = consts.tile([128, 128], BF16, tag="ident")
            make_identity(nc, ident)

            # activations that live through the attention phase
            qT_sb = acts.tile([128, NREP, T], FP16, tag="qt")
            kT_sb = acts.tile([128, T], FP16, tag="kt")
            vkd_sb = acts.tile([128, NTT, HD], BF16, tag="vkd")

            def proj_chunk(qc):
                """QKV projections + rope for token chunk qc."""
                cs = slice(512 * qc, 512 * (qc + 1))
                q_ps = [
                    psum.tile([128, 512], F32, tag="bank", name=f"qps{qc}_{h}")
                    for h in range(NREP)
                ]
                k_ps = psum.tile([128, 512], F32, tag="bank", name=f"kps{qc}")
                v_ps = psum.tile([128, 512], F32, tag="bank", name=f"vps{qc}")
                for kt in range(NKT):
                    xt = stream.tile([128, 512], FP16, tag="xt")
                    nc.sync.dma_start(
                        out=xt, in_=xT[128 * kt:128 * (kt + 1), cs]
                    )
                    st = kt == 0
                    sp = kt == NKT - 1
                    for h in range(NREP):
                        nc.tensor.matmul(
                            q_ps[h],
                            lhsT=wq_sbs[kt // 8][:, kt % 8, 128 * h:128 * (h + 1)],
                            rhs=xt,
                            start=st,
                            stop=sp,
                        )
                    nc.tensor.matmul(
                        k_ps, lhsT=wk_sb[:, kt, :], rhs=xt, start=st, stop=sp
                    )
                    nc.tensor.matmul(
                        v_ps, lhsT=wv_sb[:, kt, :], rhs=xt, start=st, stop=sp
                    )

                # rope: k first (unblocks h=0 scores), then the 4 q heads
                for h in [NREP, 0, 1, 2, 3]:
                    p = q_ps[h] if h < NREP else k_ps
                    dst = qT_sb[:, h, cs] if h < NREP else kT_sb[:, cs]
                    sw = work.tile([128, 512], F32, tag="sw")
                    nc.scalar.copy(sw[0:64, :], p[64:128, :])
                    nc.scalar.copy(sw[64:128, :], p[0:64, :])
                    rtmp = work.tile([128, 512], F32, tag="ropetmp")
                    # dst = p * cos + sw * (+-sin), fp16 conversion on the add
                    nc.vector.tensor_mul(rtmp, p, cos_sb[:, cs])
                    nc.vector.tensor_mul(sw, sw, sin_sb[:, cs])
                    nc.vector.tensor_add(dst, rtmp, sw)

                # v computed in [hd, T] layout; transpose 128x128 blocks to [k, hd]
                v_sb = work.tile([128, 512], BF16, tag="vsb")
                nc.scalar.copy(v_sb, v_ps)
                for s in range(4):
                    vt_ps = psumv.tile(
                        [128, 128], BF16, tag="vtbank", name=f"vt{qc}_{s}"
                    )
                    nc.tensor.transpose(
                        vt_ps, v_sb[:, 128 * s:128 * (s + 1)], ident
                    )
                    nc.scalar.copy(vkd_sb[:, 4 * qc + s, :], vt_ps)

            def attn_chunk(qc):
                """Causal attention for all 4 heads on chunk qc.

                The softmax finalize (denominator reduce, reciprocal,
                broadcast, normalize, store) for head h is issued after head
                h+1's score loop so its serial chain overlaps PE work; the
                last head's finalize is returned as a closure the scheduler
                places under the next chunk's projection.
                """
                cs = slice(512 * qc, 512 * (qc + 1))
                nkt = 4 * qc + 4  # causal: k tiles 0 .. 4*qc+3
                pend = []

                def finalize():
                    h, yT_ps, l_acc = pend.pop(0)
                    # denominator -> reciprocal -> broadcast to 128 partitions
                    l_bf = work.tile([128, 512], BF16, tag="lbf")
                    nc.scalar.copy(l_bf, l_acc)
                    l_ps = psum.tile([128, 512], F32, tag="bank", name=f"l{qc}{h}")
                    nc.tensor.matmul(
                        l_ps[0:1, :], lhsT=ones_col[:, 0:1], rhs=l_bf,
                        start=True, stop=True,
                    )
                    recip = lrec.tile([1, 512], F32, tag="recip")
                    nc.vector.reciprocal_approx_fast(recip, l_ps[0:1, :])
                    recip_bf = lrec.tile([1, 512], BF16, tag="recipbf")
                    nc.scalar.copy(recip_bf, recip)
                    bc_ps = psum.tile([128, 512], F32, tag="bank", name=f"b{qc}{h}")
                    nc.tensor.matmul(
                        bc_ps, lhsT=ones_row[0:1, :], rhs=recip_bf[0:1, :],
                        start=True, stop=True,
                    )
                    bc_sb = work.tile([128, 512], F32, tag="bc")
                    nc.scalar.copy(bc_sb, bc_ps)
                    yn_sb = work.tile([128, 512], FP16, tag="yn")
                    nc.vector.tensor_mul(yn_sb, yT_ps, bc_sb)
                    nc.sync.dma_start(
                        out=y_in[qc][128 * h:128 * (h + 1), :], in_=yn_sb
                    )

                for h in range(NREP):
                    yT_ps = psum.tile(
                        [128, 512], F32, tag="bank", name=f"yps{qc}_{h}"
                    )
                    l_acc = lrec.tile([128, 512], F32, tag="lacc")
                    nc.vector.memset(l_acc, 0.0)
                    for kt in range(nkt):
                        sT_ps = psum.tile(
                            [128, 512], F32, tag="bank", name=f"sps{qc}_{h}_{kt}"
                        )
                        nc.tensor.matmul(
                            sT_ps,
                            lhsT=kT_sb[:, 128 * kt:128 * (kt + 1)],
                            rhs=qT_sb[:, h, cs],
                            start=True,
                            stop=True,
                        )
                        e_sb = expp.tile([128, 512], BF16, tag="exp")
                        nc.scalar.activation(
                            e_sb, sT_ps, mybir.ActivationFunctionType.Exp,
                            scale=SCALE,
                        )
                        d = kt - 4 * qc
                        if d >= 0:  # diagonal block: zero the k > q half
                            nc.vector.tensor_mul(
                                e_sb, e_sb, mask_sb[:, 512 * d:512 * (d + 1)]
                            )
                        nc.vector.tensor_add(l_acc, l_acc, e_sb)
                        nc.tensor.matmul(
                            yT_ps,
                            lhsT=vkd_sb[:, kt, :],
                            rhs=e_sb,
                            start=(kt == 0),
                            stop=(kt == nkt - 1),
                        )
                    pend.append((h, yT_ps, l_acc))
                    if h > 0:
                        finalize()  # finalize head h-1 under head h's PE work
                return finalize
            def gather_chunk(qc):
                nc.gpsimd.collective_compute(
                    "AllGather",
                    mybir.AluOpType.bypass,
                    ins=[y_in[qc][:, :]],
                    outs=[y_all[qc][:, :]],
                    replica_groups=[list(range(N_CORES))],
                )

            def wo_chunk(qc):
                """out rows for chunk qc: needs y_all[qc] (all cores' heads)."""
                y_sb = ybuf.tile([128, NKT, 512], FP16, tag="ysb")
                y_r = y_all[qc].rearrange("(n p) m -> p n m", p=128)
                for s in range(8):
                    nc.gpsimd.dma_start(
                        out=y_sb[:, 4 * s:4 * (s + 1), :],
                        in_=y_r[:, 4 * s:4 * (s + 1), :],
                    )
                for tt in range(4 * qc, 4 * qc + 4):
                    to = 128 * tt - 512 * qc
                    o_ps = psum.tile([128, 512], F32, tag="bank", name=f"o{tt}")
                    for kt in range(NKT):
                        nc.tensor.matmul(
                            o_ps,
                            lhsT=y_sb[:, kt, to:to + 128],
                            rhs=wo_sb[:, kt, :],
                            start=(kt == 0),
                            stop=(kt == NKT - 1),
                        )
                    o_sb = outp.tile([128, 512], F32, tag="osb")
                    nc.scalar.copy(o_sb, o_ps)
                    nc.sync.dma_start(
                        out=out[128 * tt:128 * (tt + 1), :], in_=o_sb
                    )

            # ---------- chunk-pipelined schedule ----------
            # wo(qc) is issued two chunks late so the AllGather latency is
            # covered by proj/attn of the following chunks.
            proj_chunk(0)
            fin0 = attn_chunk(0)
            wo_r = wo_p.rearrange("p (n m) -> p n m", n=NKT)
            for s in range(4):
                nc.gpsimd.dma_start(
                    out=wo_sb[:, 8 * s:8 * (s + 1), :],
                    in_=wo_r[:, 8 * s:8 * (s + 1), :],
                )
            proj_chunk(1)
            fin0()
            gather_chunk(0)
            fin1 = attn_chunk(1)
            proj_chunk(2)
            fin1()
            gather_chunk(1)
            wo_chunk(0)
            fin2 = attn_chunk(2)
            proj_chunk(3)
            fin2()
            gather_chunk(2)
            wo_chunk(1)
            fin3 = attn_chunk(3)
            wo_chunk(2)
            fin3()
            gather_chunk(3)
            wo_chunk(3)

    nc.compile()
    _cached["nc"] = nc
    return nc


def _build_in_maps(inputs):
    return _shard_inputs(**inputs)


def _pack_w(wT, m):
    """[DIM, m] -> [128, NKT*m] with each partition's rows DRAM-contiguous."""
    return np.ascontiguousarray(
        wT.reshape(NKT, 128, m).transpose(1, 0, 2).reshape(128, NKT * m)
    )


def _shard_inputs(x, cos, sin, wq, wk, wv, wo, start_pos):
    import ml_dtypes

    bf16 = ml_dtypes.bfloat16
    x = np.asarray(x, dtype=np.float32)
    cos = np.asarray(cos, dtype=np.float32)
    sin = np.asarray(sin, dtype=np.float32)
    wq = np.asarray(wq, dtype=np.float32)
    wk = np.asarray(wk, dtype=np.float32)
    wv = np.asarray(wv, dtype=np.float32)
    wo = np.asarray(wo, dtype=np.float32)
    sp = int(start_pos)

    xT = np.ascontiguousarray(x[0].T).astype(np.float16)  # (DIM, T)
    cosT = np.ascontiguousarray(cos[sp:sp + T].T)  # (64, T)
    sinT = np.ascontiguousarray(sin[sp:sp + T].T)
    cos2 = np.concatenate([cosT, cosT], axis=0)  # (128, T)
    sin2 = np.concatenate([-sinT, sinT], axis=0)  # rotate-half signs folded in

    kk = np.arange(128)[:, None]
    qq = np.arange(512)[None, :]
    masks = np.concatenate(
        [(kk + 128 * d <= qq).astype(np.float32) for d in range(4)], axis=1
    ).astype(bf16)  # (128, 2048)

    in_maps = []
    for c in range(N_CORES):
        qrows = slice(NREP * HD * c, NREP * HD * (c + 1))
        krows = slice(HD * c, HD * (c + 1))
        in_maps.append({
            "xT": xT,
            "cos2": cos2,
            "sin2": sin2,
            "masks": masks,
            "wq_p": _pack_w(wq[qrows, :].T.astype(np.float16), NREP * HD),
            "wk_p": _pack_w(wk[krows, :].T.astype(np.float16), HD),
            "wv_p": _pack_w(wv[krows, :].T.astype(np.float16), HD),
            "wo_p": _pack_w(wo[qrows, :].T.astype(np.float16), NREP * HD),
        })
    return in_maps


def kernel(x, cos, sin, wq, wk, wv, wo, start_pos):
    in_maps = _shard_inputs(x, cos, sin, wq, wk, wv, wo, start_pos)
    nc = _build_kernel()
    res = run_bass_kernel_spmd(nc, in_maps, core_ids=list(range(N_CORES)))
    out = np.concatenate([res.results[c]["out"] for c in range(N_CORES)], axis=1)
    return out.reshape(1, T, DIM).astype(np.float32)


# revision 19
# speedup vs baseline: 1.0833x; 1.0330x over previous
"""GroupedQueryAttention (B=1, T=2048, D=4096, 32 q-heads / 8 kv-heads, hd=128)
on 8 trn2 NeuronCores.

Sharding: kv-head parallel — core c owns kv head c and its 4 query heads.
v3: 16-bit matmuls (fp16 on the q/k score path and projections for mantissa,
bf16 on the exp/value path for range), chunk-pipelined schedule with one
AllGather per 512-token chunk overlapped two chunks deep, wo matmul
column-parallel so no AllReduce is needed.  Causal attention in transposed
[k, q] score layout (softmax without max-subtraction: fp32 exp can't
overflow at these score magnitudes).
"""
import sys

sys.path.insert(0, "/opt/trn_rl_repo")

import numpy as np

import concourse.bacc as bacc
import concourse.tile as tile
from concourse import mybir
from concourse.bass_utils import run_bass_kernel_spmd
from concourse.masks import make_identity

N_CORES = 8
T = 2048
DIM = 4096
HD = 128
NH = 32
NKV = 8
NREP = NH // NKV  # 4 query heads per core
NCHUNK = T // 512  # 4 chunks of 512 along T
NKT = DIM // 128  # 32 contraction tiles for the projections
NTT = T // 128  # 16 row tiles for the wo matmul
F32 = mybir.dt.float32
FP16 = mybir.dt.float16
BF16 = mybir.dt.bfloat16
SCALE = 1.0 / float(np.sqrt(HD))

_cached = {}


def _build_kernel():
    if "nc" in _cached:
        return _cached["nc"]

    nc = bacc.Bacc("TRN2", target_bir_lowering=False)

    xT = nc.dram_tensor("xT", [DIM, T], FP16, kind="ExternalInput")
    cos2 = nc.dram_tensor("cos2", [128, T], F32, kind="ExternalInput")
    sin2 = nc.dram_tensor("sin2", [128, T], F32, kind="ExternalInput")
    masks = nc.dram_tensor("masks", [128, 4 * 512], BF16, kind="ExternalInput")
    # weights pre-packed on host: [128, n*m] with partition-contiguous rows
    wq_p = nc.dram_tensor("wq_p", [128, NKT * NREP * HD], FP16, kind="ExternalInput")
    wk_p = nc.dram_tensor("wk_p", [128, NKT * HD], FP16, kind="ExternalInput")
    wv_p = nc.dram_tensor("wv_p", [128, NKT * HD], FP16, kind="ExternalInput")
    wo_p = nc.dram_tensor("wo_p", [128, NKT * NREP * HD], FP16, kind="ExternalInput")
    out = nc.dram_tensor("out", [T, NREP * HD], F32, kind="ExternalOutput")

    y_in = [
        nc.dram_tensor(f"y_in{qc}", [NREP * HD, 512], FP16, kind="Internal")
        for qc in range(NCHUNK)
    ]
    y_all = [
        nc.dram_tensor(
            f"y_all{qc}", [DIM, 512], FP16, kind="Internal", addr_space="Shared"
        )
        for qc in range(NCHUNK)
    ]

    with tile.TileContext(nc) as tc:
        with (
            tc.tile_pool(name="consts", bufs=1) as consts,
            tc.tile_pool(name="weights", bufs=1) as weights,
            tc.tile_pool(name="acts", bufs=1) as acts,
            tc.tile_pool(name="ybuf", bufs=1) as ybuf,
            tc.tile_pool(name="stream", bufs=8) as stream,
            tc.tile_pool(name="work", bufs=2) as work,
            tc.tile_pool(name="lrec", bufs=2) as lrec,
            tc.tile_pool(name="expp", bufs=6) as expp,
            tc.tile_pool(name="outp", bufs=2) as outp,
            tc.tile_pool(name="psum", bufs=7, space="PSUM") as psum,
            tc.tile_pool(name="psumv", bufs=1, space="PSUM") as psumv,
        ):
            # ---------- resident weights, kt-sliced so proj(0) starts early ----
            wq_r = wq_p.rearrange("p (n m) -> p n m", n=NKT)
            wq_sbs = []
            wq_sbs.append(weights.tile([128, 8, NREP * HD], FP16, tag="wq0", name="wq_t0"))
            wk_sb = weights.tile([128, NKT, HD], FP16, tag="wk")
            wv_sb = weights.tile([128, NKT, HD], FP16, tag="wv")
            wk_r = wk_p.rearrange("p (n m) -> p n m", n=NKT)
            wv_r = wv_p.rearrange("p (n m) -> p n m", n=NKT)
            for j in range(2):
                nc.gpsimd.dma_start(
                    out=wq_sbs[0][:, 4 * j:4 * (j + 1), :],
                    in_=wq_r[:, 4 * j:4 * (j + 1), :],
                )
            nc.gpsimd.dma_start(out=wk_sb, in_=wk_r)
            nc.gpsimd.dma_start(out=wv_sb, in_=wv_r)
            for s in range(1, 4):
                wq_sbs.append(
                    weights.tile(
                        [128, 8, NREP * HD], FP16, tag=f"wq{s}", name=f"wq_t{s}"
                    )
                )
                nc.gpsimd.dma_start(
                    out=wq_sbs[s], in_=wq_r[:, 8 * s:8 * (s + 1), :]
                )
            wo_sb = weights.tile([128, NKT, NREP * HD], FP16, tag="wo")

            # ---------- constants (needed only once rope starts) ----------
            cos_sb = consts.tile([128, T], F32, tag="cos")
            nc.scalar.dma_start(out=cos_sb, in_=cos2[:, :])
            sin_sb = consts.tile([128, T], F32, tag="sin")
            nc.scalar.dma_start(out=sin_sb, in_=sin2[:, :])
            mask_sb = consts.tile([128, 4 * 512], BF16, tag="mask")
            nc.scalar.dma_start(out=mask_sb, in_=masks[:, :])
            ones_col = consts.tile([128, 1], BF16, tag="onesc")
            nc.vector.memset(ones_col, 1.0)
            ones_row = consts.tile([1, 128], BF16, tag="onesr")
            nc.vector.memset(ones_row, 1.0)
            ident = consts.tile([128, 128], BF16, tag="ident")
            make_identity(nc, ident)

            warm_sb = consts.tile([128, 64], FP16, tag="warm")
            nc.vector.memset(warm_sb, 0.0)

            def pe_warm(n):
                """Dummy matmuls to keep the PE HAM clock warm during waits."""
                w_ps = psumv.tile([64, 64], F32, tag="vtbank", name=f"wp{pe_warm.i}")
                pe_warm.i += 1
                for i in range(n):
                    nc.tensor.matmul(
                        w_ps, lhsT=warm_sb[:, 0:64], rhs=warm_sb,
                        start=(i == 0), stop=(i == n - 1),
                        skip_group_check=True,
                    )
            pe_warm.i = 0

            pe_warm(150)

            # activations that live through the attention phase
            qT_sb = acts.tile([128, NREP, T], FP16, tag="qt")
            kT_sb = acts.tile([128, T], FP16, tag="kt")
            vkd_sb = acts.tile([128, NTT, HD], BF16, tag="vkd")

            def proj_chunk(qc):
                """QKV projections + rope for token chunk qc."""
                cs = slice(512 * qc, 512 * (qc + 1))
                q_ps = [
                    psum.tile([128, 512], F32, tag="bank", name=f"qps{qc}_{h}")
                    for h in range(NREP)
                ]
                k_ps = psum.tile([128, 512], F32, tag="bank", name=f"kps{qc}")
                v_ps = psum.tile([128, 512], F32, tag="bank", name=f"vps{qc}")
                for kt in range(NKT):
                    xt = stream.tile([128, 512], FP16, tag="xt")
                    nc.sync.dma_start(
                        out=xt, in_=xT[128 * kt:128 * (kt + 1), cs]
                    )
                    st = kt == 0
                    sp = kt == NKT - 1
                    for h in range(NREP):
                        nc.tensor.matmul(
                            q_ps[h],
                            lhsT=wq_sbs[kt // 8][:, kt % 8, 128 * h:128 * (h + 1)],
                            rhs=xt,
                            start=st,
                            stop=sp,
                        )
                    nc.tensor.matmul(
                        k_ps, lhsT=wk_sb[:, kt, :], rhs=xt, start=st, stop=sp
                    )
                    nc.tensor.matmul(
                        v_ps, lhsT=wv_sb[:, kt, :], rhs=xt, start=st, stop=sp
                    )

                # rope: k first (unblocks h=0 scores), then the 4 q heads
                for h in [NREP, 0, 1, 2, 3]:
                    p = q_ps[h] if h < NREP else k_ps
                    dst = qT_sb[:, h, cs] if h < NREP else kT_sb[:, cs]
                    sw = work.tile([128, 512], F32, tag="sw")
                    nc.scalar.copy(sw[0:64, :], p[64:128, :])
                    nc.scalar.copy(sw[64:128, :], p[0:64, :])
                    rtmp = work.tile([128, 512], F32, tag="ropetmp")
                    # dst = p * cos + sw * (+-sin), fp16 conversion on the add
                    nc.vector.tensor_mul(rtmp, p, cos_sb[:, cs])
                    nc.vector.tensor_mul(sw, sw, sin_sb[:, cs])
                    nc.vector.tensor_add(dst, rtmp, sw)

                # v computed in [hd, T] layout; transpose 128x128 blocks to [k, hd]
                v_sb = work.tile([128, 512], BF16, tag="vsb")
                nc.scalar.copy(v_sb, v_ps)
                for s in range(4):
                    vt_ps = psumv.tile(
                        [128, 128], BF16, tag="vtbank", name=f"vt{qc}_{s}"
                    )
                    nc.tensor.transpose(
                        vt_ps, v_sb[:, 128 * s:128 * (s + 1)], ident
                    )
                    nc.scalar.copy(vkd_sb[:, 4 * qc + s, :], vt_ps)

            def attn_chunk(qc):
                """Causal attention for all 4 heads on chunk qc.

                The softmax finalize (denominator reduce, reciprocal,
                broadcast, normalize, store) for head h is issued after head
                h+1's score loop so its serial chain overlaps PE work; the
                last head's finalize is returned as a closure the scheduler
                places under the next chunk's projection.
                """
                cs = slice(512 * qc, 512 * (qc + 1))
                nkt = 4 * qc + 4  # causal: k tiles 0 .. 4*qc+3
                pendA = []
                pendB = []

                def stage_a():
                    # free the PSUM bank, reduce denominator, reciprocal
                    h, yT_ps, l_acc = pendA.pop(0)
                    yT_sb = work.tile([128, 512], F32, tag="ytsb")
                    nc.scalar.copy(yT_sb, yT_ps)
                    l_bf = work.tile([128, 512], BF16, tag="lbf")
                    nc.scalar.copy(l_bf, l_acc)
                    l_ps = psum.tile([128, 512], F32, tag="bank", name=f"l{qc}{h}")
                    nc.tensor.matmul(
                        l_ps[0:1, :], lhsT=ones_col[:, 0:1], rhs=l_bf,
                        start=True, stop=True,
                    )
                    recip = lrec.tile([1, 512], F32, tag="recip")
                    nc.vector.reciprocal_approx_fast(recip, l_ps[0:1, :])
                    recip_bf = lrec.tile([1, 512], BF16, tag="recipbf")
                    nc.scalar.copy(recip_bf, recip)
                    pendB.append((h, yT_sb, recip_bf))

                def stage_b():
                    # broadcast 1/l to 128 partitions, normalize, store
                    h, yT_sb, recip_bf = pendB.pop(0)
                    bc_ps = psum.tile([128, 512], F32, tag="bank", name=f"b{qc}{h}")
                    nc.tensor.matmul(
                        bc_ps, lhsT=ones_row[0:1, :], rhs=recip_bf[0:1, :],
                        start=True, stop=True,
                    )
                    bc_sb = work.tile([128, 512], F32, tag="bc")
                    nc.scalar.copy(bc_sb, bc_ps)
                    yn_sb = work.tile([128, 512], FP16, tag="yn")
                    nc.vector.tensor_mul(yn_sb, yT_sb, bc_sb)
                    nc.sync.dma_start(
                        out=y_in[qc][128 * h:128 * (h + 1), :], in_=yn_sb
                    )

                for h in range(NREP):
                    yT_ps = psum.tile(
                        [128, 512], F32, tag="bank", name=f"yps{qc}_{h}"
                    )
                    l_acc = lrec.tile([128, 512], F32, tag="lacc")
                    nc.vector.memset(l_acc, 0.0)
                    for kt in range(nkt):
                        sT_ps = psum.tile(
                            [128, 512], F32, tag="bank", name=f"sps{qc}_{h}_{kt}"
                        )
                        nc.tensor.matmul(
                            sT_ps,
                            lhsT=kT_sb[:, 128 * kt:128 * (kt + 1)],
                            rhs=qT_sb[:, h, cs],
                            start=True,
                            stop=True,
                        )
                        e_sb = expp.tile([128, 512], BF16, tag="exp")
                        nc.scalar.activation(
                            e_sb, sT_ps, mybir.ActivationFunctionType.Exp,
                            scale=SCALE,
                        )
                        d = kt - 4 * qc
                        if d >= 0:  # diagonal block: zero the k > q half
                            nc.vector.tensor_mul(
                                e_sb, e_sb, mask_sb[:, 512 * d:512 * (d + 1)]
                            )
                        nc.vector.tensor_add(l_acc, l_acc, e_sb)
                        nc.tensor.matmul(
                            yT_ps,
                            lhsT=vkd_sb[:, kt, :],
                            rhs=e_sb,
                            start=(kt == 0),
                            stop=(kt == nkt - 1),
                        )
                    pendA.append((h, yT_ps, l_acc))
                    if h >= 1:
                        stage_a()  # head h-1, under head h's PE work
                    if h >= 2:
                        stage_b()  # head h-2

                def rest():
                    stage_a()
                    stage_b()
                    stage_b()
                return rest
            def gather_chunk(qc):
                nc.gpsimd.collective_compute(
                    "AllGather",
                    mybir.AluOpType.bypass,
                    ins=[y_in[qc][:, :]],
                    outs=[y_all[qc][:, :]],
                    replica_groups=[list(range(N_CORES))],
                )

            def wo_chunk(qc):
                """out rows for chunk qc: needs y_all[qc] (all cores' heads)."""
                y_sb = ybuf.tile([128, NKT, 512], FP16, tag="ysb")
                y_r = y_all[qc].rearrange("(n p) m -> p n m", p=128)
                for s in range(8):
                    nc.gpsimd.dma_start(
                        out=y_sb[:, 4 * s:4 * (s + 1), :],
                        in_=y_r[:, 4 * s:4 * (s + 1), :],
                    )
                for tt in range(4 * qc, 4 * qc + 4):
                    to = 128 * tt - 512 * qc
                    o_ps = psum.tile([128, 512], F32, tag="bank", name=f"o{tt}")
                    for kt in range(NKT):
                        nc.tensor.matmul(
                            o_ps,
                            lhsT=y_sb[:, kt, to:to + 128],
                            rhs=wo_sb[:, kt, :],
                            start=(kt == 0),
                            stop=(kt == NKT - 1),
                        )
                    o_sb = outp.tile([128, 512], F32, tag="osb")
                    nc.scalar.copy(o_sb, o_ps)
                    nc.sync.dma_start(
                        out=out[128 * tt:128 * (tt + 1), :], in_=o_sb
                    )

            # ---------- chunk-pipelined schedule ----------
            # wo(qc) is issued two chunks late so the AllGather latency is
            # covered by proj/attn of the following chunks.
            proj_chunk(0)
            fin0 = attn_chunk(0)
            wo_r = wo_p.rearrange("p (n m) -> p n m", n=NKT)
            for s in range(4):
                nc.gpsimd.dma_start(
                    out=wo_sb[:, 8 * s:8 * (s + 1), :],
                    in_=wo_r[:, 8 * s:8 * (s + 1), :],
                )
            proj_chunk(1)
            fin0()
            gather_chunk(0)
            fin1 = attn_chunk(1)
            proj_chunk(2)
            fin1()
            gather_chunk(1)
            wo_chunk(0)
            fin2 = attn_chunk(2)
            proj_chunk(3)
            fin2()
            gather_chunk(2)
            wo_chunk(1)
            fin3 = attn_chunk(3)
            wo_chunk(2)
            fin3()
            gather_chunk(3)
            pe_warm(250)
            wo_chunk(3)

    nc.compile()
    _cached["nc"] = nc
    return nc


def _build_in_maps(inputs):
    return _shard_inputs(**inputs)


def _pack_w(wT, m):
    """[DIM, m] -> [128, NKT*m] with each partition's rows DRAM-contiguous."""
    return np.ascontiguousarray(
        wT.reshape(NKT, 128, m).transpose(1, 0, 2).reshape(128, NKT * m)
    )


def _shard_inputs(x, cos, sin, wq, wk, wv, wo, start_pos):
    import ml_dtypes

    bf16 = ml_dtypes.bfloat16
    x = np.asarray(x, dtype=np.float32)
    cos = np.asarray(cos, dtype=np.float32)
    sin = np.asarray(sin, dtype=np.float32)
    wq = np.asarray(wq, dtype=np.float32)
    wk = np.asarray(wk, dtype=np.float32)
    wv = np.asarray(wv, dtype=np.float32)
    wo = np.asarray(wo, dtype=np.float32)
    sp = int(start_pos)

    xT = np.ascontiguousarray(x[0].T).astype(np.float16)  # (DIM, T)
    cosT = np.ascontiguousarray(cos[sp:sp + T].T)  # (64, T)
    sinT = np.ascontiguousarray(sin[sp:sp + T].T)
    cos2 = np.concatenate([cosT, cosT], axis=0)  # (128, T)
    sin2 = np.concatenate([-sinT, sinT], axis=0)  # rotate-half signs folded in

    kk = np.arange(128)[:, None]
    qq = np.arange(512)[None, :]
    masks = np.concatenate(
        [(kk + 128 * d <= qq).astype(np.float32) for d in range(4)], axis=1
    ).astype(bf16)  # (128, 2048)

    in_maps = []
    for c in range(N_CORES):
        qrows = slice(NREP * HD * c, NREP * HD * (c + 1))
        krows = slice(HD * c, HD * (c + 1))
        in_maps.append({
            "xT": xT,
            "cos2": cos2,
            "sin2": sin2,
            "masks": masks,
            "wq_p": _pack_w(wq[qrows, :].T.astype(np.float16), NREP * HD),
            "wk_p": _pack_w(wk[krows, :].T.astype(np.float16), HD),
            "wv_p": _pack_w(wv[krows, :].T.astype(np.float16), HD),
            "wo_p": _pack_w(wo[qrows, :].T.astype(np.float16), NREP * HD),
        })
    return in_maps


def kernel(x, cos, sin, wq, wk, wv, wo, start_pos):
    in_maps = _shard_inputs(x, cos, sin, wq, wk, wv, wo, start_pos)
    nc = _build_kernel()
    res = run_bass_kernel_spmd(nc, in_maps, core_ids=list(range(N_CORES)))
    out = np.concatenate([res.results[c]["out"] for c in range(N_CORES)], axis=1)
    return out.reshape(1, T, DIM).astype(np.float32)
